# revision 16
# baseline (speedup 1.0000x reference)
"""Quanvolutional layer (nn_ConvGenQuantum) as a Trainium2 Bass kernel.

The reference applies, per 2x2 image patch (p0,p1,p2,p3), a fixed 4-qubit
circuit: RY(p_w) encoders, then a fixed 8-gate random layer with params
theta[0..4], then measures <Z_w>. Conjugating each Z_w through the circuit
(Heisenberg picture) and dropping Pauli strings containing Y (the encoded
state is real, so those have zero expectation) collapses the whole circuit
to a closed form:

    q0 = cos(p0 + theta0); q1 = cos(p1); q2 = cos(p2); q3 = cos(p3 + theta3)
    E0 = cos(theta4) * q0
    E1 = cos(theta1) * q0 * q1
    E2 = E1 * q2
    E3 = E2 * q3

(theta2 -- the RZ -- drops out entirely.)

cos is evaluated via the half-angle identity cos(a) = 1 - 2*sin(a/2)^2.

Device-side data layout (v4): the host de-interleaves each image's 2x2
patches into four contiguous 196-pixel PLANES, folds the per-plane angle
offsets (theta0 / theta3-pi) into the pixels, wraps every angle into
[-pi, pi], pads each plane to 224 elements (64B-aligned slices) and narrows
to fp16. Outputs are produced as four contiguous E-planes in fp16 and
re-interleaved/upcast by the host. Every engine op is contiguous packed
fp16 (DVE 2x/4x perf modes), and all four planes share Sin bias 0 so the
encoder is ONE ScalarE Sin per chunk. Per 128-row chunk:

    u    = Sin(0.5*x)                  ScalarE, one op over all 4 planes
    sq3  = u3*u3;  n3 = 2*sq3 - 1      GpSimd (plane 3; = +cos(p3+th3))
    sq   = u012^2                      DVE tensor_scalar pow
    n0   = s1 - 2*s1*sq0               DVE tensor_scalar
    n12  = 1 - 2*sq12                  DVE tensor_scalar
    E0   = s4 - 2*s4*sq0               DVE tensor_scalar
    E1   = n0*n1;  b = n2*n3           DVE tensor_tensor
    E2   = E1*n2;  E3 = E1*b           DVE tensor_tensor

Chunks are software-pipelined (stage A: DMA+Sin+squares, stage B: products
+ output DMA, emitted A0 A1 B0 A2 B1 A3 B2 B3) so dependent DVE ops are
never back-to-back (write->read pipeline bubbles). Batch is sharded
4096/8 = 512 images per NeuronCore, pure data parallel, no collectives.
"""

import numpy as np

import concourse.bass as bass
import concourse.bacc as bacc
import concourse.tile as tile
from concourse import mybir
from concourse.bass_utils import run_bass_kernel_spmd

F16 = mybir.dt.float16
F32 = mybir.dt.float32
N_CORES = 8
B_TOTAL = 4096
ROWS = B_TOTAL // N_CORES       # images per core
Q = 196                         # patches per image
QP = 224                        # padded plane stride (448B, 64B-aligned)
PIXP = 4 * QP                   # padded pixels per image
N_CHUNKS = 4
POOL_PLANE3 = True              # plane-3 square/affine on GpSimd vs DVE

LAST_RESULT = None              # BassKernelResults of the most recent run


def _drain_and_single_barrier(self, tick_clock, wait_clock):
    """TileContext exit without the two tile barriers: the semaphore clear
    between them is already skipped (runtime resets semaphores), and the
    bacc epilogue emits its own all-engine rendezvous, so the sync-engine
    drain (which waits every tile semaphore at its final value, including
    the output-DMA completions) is sufficient here."""
    drain_inst = self.nc.sync.drain()
    wait_clock.add_sem_waits(
        drain_inst.ins, tile.ScopedClock({None: tick_clock.global_clock})
    )
    popped = self.nc._tile_sem_poison_stack.pop()
    assert popped is self._sem_poison


def _build(th1: float, th4: float):
    """Per-core Bass program: [ROWS, PIXP] fp16 plane-major wrapped angles
    -> [ROWS, PIXP] fp16 plane-major expectations."""
    orig_barrier = bass.Bass.all_engine_barrier
    bass.Bass.all_engine_barrier = lambda self, **kw: None
    try:
        nc = bacc.Bacc(None, target_bir_lowering=False, debug=False)
    finally:
        bass.Bass.all_engine_barrier = orig_barrier

    nc.clear_and_free_semaphores = lambda sems: None

    s1 = float(np.cos(th1))
    s4 = float(np.cos(th4))

    x = nc.declare_dram_parameter("x", [ROWS, PIXP], F16, isOutput=False)
    out = nc.declare_dram_parameter("out", [ROWS, PIXP], F16, isOutput=True)

    add = mybir.AluOpType.add
    mult = mybir.AluOpType.mult
    pow_ = mybir.AluOpType.pow
    SIN = mybir.ActivationFunctionType.Sin

    def planes(t, lo, hi):
        # [128, planes hi-lo, 196] view skipping the 28-element pad lanes
        return t.rearrange("p (w q) -> p w q", q=QP)[:, lo:hi, 0:Q]

    state = {}

    def stage_a(c, io_pool, q_pool):
        r0 = c * 128
        xt = io_pool.tile([128, PIXP], F16, tag=f"x{c}")
        nc.sync.dma_start(out=xt[:, :], in_=x[r0:r0 + 128, :])

        ua = q_pool.tile([128, PIXP], F16, tag="ua")
        nc.scalar.activation(planes(ua, 0, 4), planes(xt, 0, 4), SIN,
                             bias=0.0, scale=0.5)

        if POOL_PLANE3:
            # plane 3 on GpSimd in private tiles
            sq3 = q_pool.tile([128, QP], F16, tag="sq3")
            nc.gpsimd.tensor_tensor(sq3[:, 0:Q], planes(ua, 3, 4)[:, 0, :],
                                    planes(ua, 3, 4)[:, 0, :], op=mult)
            n3 = q_pool.tile([128, QP], F16, tag="n3")
            nc.gpsimd.tensor_scalar(n3[:, 0:Q], sq3[:, 0:Q],
                                    -2.0, 1.0, op0=mult, op1=add)
            sq = q_pool.tile([128, 3 * QP], F16, tag="sq")
            nc.vector.tensor_tensor(planes(sq, 0, 3), planes(ua, 0, 3),
                                    planes(ua, 0, 3), op=mult)
        else:
            n3 = None
            sq = q_pool.tile([128, PIXP], F16, tag="sq")
            nc.vector.tensor_tensor(planes(sq, 0, 4), planes(ua, 0, 4),
                                    planes(ua, 0, 4), op=mult)
        state[c] = (xt, sq, n3)

    def stage_b(c, io_pool, q_pool):
        r0 = c * 128
        _, sq, n3 = state.pop(c)
        sq0 = sq.rearrange("p (w q) -> p w q", q=QP)[:, 0, 0:Q]

        # nt planes: n0 | n1 | n2 | n3-or-unused | b
        nt = q_pool.tile([128, 5 * QP], F16, tag="nt")
        ntv = nt.rearrange("p (w q) -> p w q", q=QP)
        nc.vector.tensor_scalar(ntv[:, 0, 0:Q], sq0,
                                -2.0 * s1, s1, op0=mult, op1=add)
        if n3 is None:
            # n1,n2,n3 in one 4x tensor_scalar into nt planes 1-3
            nc.vector.tensor_scalar(ntv[:, 1:4, 0:Q], planes(sq, 1, 4),
                                    -2.0, 1.0, op0=mult, op1=add)
            n3v = ntv[:, 3, 0:Q]
        else:
            nc.vector.tensor_scalar(ntv[:, 1:3, 0:Q], planes(sq, 1, 3),
                                    -2.0, 1.0, op0=mult, op1=add)
            n3v = n3[:, 0:Q]

        ot = io_pool.tile([128, PIXP], F16, tag=f"o{c}")
        otv = ot.rearrange("p (w q) -> p w q", q=QP)
        # nt plane 3 holds b = n2*n3 so [n2|b] is one contiguous view
        # E0 = s4*m0; E1 = n0*n1; b = n2*n3; (E2,E3) = (n2,b)*E1
        # E0 is a pure affine of sq0, so it runs on the Scalar engine
        # (Copy = scale*x + bias), which has idle capacity after the Sins.
        COPY = mybir.ActivationFunctionType.Copy
        nc.scalar.activation(otv[:, 0, 0:Q], sq0, COPY,
                             bias=s4, scale=-2.0 * s4)
        nc.vector.tensor_tensor(otv[:, 1, 0:Q], ntv[:, 0, 0:Q],
                                ntv[:, 1, 0:Q], op=mult)
        nc.vector.tensor_tensor(ntv[:, 4, 0:Q], ntv[:, 2, 0:Q],
                                n3v, op=mult)
        nc.vector.tensor_tensor(otv[:, 2, 0:Q], otv[:, 1, 0:Q],
                                ntv[:, 2, 0:Q], op=mult)
        nc.vector.tensor_tensor(otv[:, 3, 0:Q], otv[:, 1, 0:Q],
                                ntv[:, 4, 0:Q], op=mult)

        if c == N_CHUNKS - 1:
            # split the last chunk's output so planes 0-1 ship while
            # (E2,E3) still compute: shorter exposed drain
            nc.sync.dma_start(out=out[r0:r0 + 128, 0:2 * QP],
                              in_=ot[:, 0:2 * QP])
            nc.sync.dma_start(out=out[r0:r0 + 128, 2 * QP:],
                              in_=ot[:, 2 * QP:])
        else:
            nc.sync.dma_start(out=out[r0:r0 + 128, :], in_=ot[:, :])

    with tile.TileContext(nc) as tc:
        tc._drain_and_barrier = _drain_and_single_barrier.__get__(tc)
        with tc.tile_pool(name="io", bufs=2) as io_pool, \
             tc.tile_pool(name="qp", bufs=2) as q_pool:
            t = nc.alloc_sbuf_tensor("const-zero", [128, 1], F32)
            nc.gpsimd.memset(t.ap(), 0.0)
            nc.const_aps.aps[(F32, 0.0)] = t.ap()

            # Dummy activation so the ACT table load (~1.3us) overlaps the
            # input DMA instead of blocking the first real Sin.
            warm = nc.alloc_sbuf_tensor("act-warm", [128, 1], F32)
            nc.scalar.activation(warm.ap(), nc.const_aps.aps[(F32, 0.0)],
                                 SIN, bias=0.0, scale=1.0)

            # software pipeline: A0 A1 B0 A2 B1 A3 B2 B3
            stage_a(0, io_pool, q_pool)
            for c in range(1, N_CHUNKS):
                stage_a(c, io_pool, q_pool)
                stage_b(c - 1, io_pool, q_pool)
            stage_b(N_CHUNKS - 1, io_pool, q_pool)

    if not nc.is_finalized():
        nc.finalize()
    return nc


def kernel(x: np.ndarray, theta: np.ndarray, _trace: bool = False) -> np.ndarray:
    global LAST_RESULT
    th = np.asarray(theta, dtype=np.float64)
    nc = _build(th1=float(th[1]), th4=float(th[4]))

    # Host-side marshalling: de-interleave 2x2 patches into plane-major
    # order (pixel (2a+b, 2c+d) -> plane 2b+d, patch a*14+c), fold the
    # plane angle offsets into the data, wrap into [-pi, pi], pad planes
    # to 224 and narrow to fp16.
    xf = np.asarray(x, dtype=np.float32).reshape(B_TOTAL, 14, 2, 14, 2)
    xf = xf.transpose(0, 2, 4, 1, 3).reshape(B_TOTAL, 4, Q).copy()
    xf[:, 0, :] += np.float32(th[0])
    xf[:, 3, :] += np.float32(th[3])
    two_pi = np.float32(2 * np.pi)
    xf -= two_pi * np.round(xf / two_pi)
    xh = np.zeros((B_TOTAL, 4, QP), np.float16)
    xh[:, :, 0:Q] = xf
    xh = np.ascontiguousarray(xh.reshape(B_TOTAL, PIXP))

    in_maps = [{"x": xh[i * ROWS:(i + 1) * ROWS]} for i in range(N_CORES)]
    res = run_bass_kernel_spmd(nc, in_maps, core_ids=list(range(N_CORES)),
                               trace=_trace)
    LAST_RESULT = res
    oh = np.concatenate([res.results[i]["out"] for i in range(N_CORES)],
                        axis=0)
    # Re-interleave E-planes into per-patch (E0,E1,E2,E3) order and upcast.
    o = oh.reshape(B_TOTAL, 4, QP)[:, :, 0:Q].transpose(0, 2, 1)
    return np.ascontiguousarray(o.astype(np.float32).reshape(B_TOTAL, 4 * Q))
